# revision 1
# baseline (speedup 1.0000x reference)
"""CondLaneHead DynamicMaskHead kernel for 8 Trainium2 NeuronCores.

Problem: per-instance 3-layer 1x1-conv MLP over a [64,160,256] feature map.
  feats = concat([loc_x, loc_y], x[img])            # [66, L], L = 160*256
  h1 = relu(w0 @ feats + b0)                        # [64, L]
  h2 = relu(w1 @ h1 + b1)                           # [64, L]
  out = w2 @ h2 + b2 - 2.19                         # [1, L]
32 instances (8 per image, 4 images). Sharding: 4 instances per core; each
core needs exactly one image's feature map.

Device mapping (per core):
  - The 4 instances form 2 pairs. Layer 1: one matmul per pair with
    lhsT = [67, 128] (2 coord rows + ones row for the bias + 64 x rows).
    Layer 2: block-diagonal [128, 128] weights, one matmul per pair.
  - Layer 3 (64->1): output is packed across PSUM partitions. Matmuls write
    [32, 512] windows at partition bases 0/32/64/96 using zero-padded weight
    variants (w2 placed at columns 2j of window j), accumulating over 16
    position-groups per window, so one PSUM bank collects 64 groups x 2
    instances = a full [128, 512] tile before a single bias+copy op and one
    scatter-DMA to DRAM.
  - dtypes: layer 1 runs in float32r (full-rate fp32 storage on the PE),
    h1/h2 and layer-2/3 weights in bf16, all PSUM accumulation in fp32.
"""

import sys

if "/opt/trn_rl_repo" not in sys.path:
    sys.path.insert(0, "/opt/trn_rl_repo")

import numpy as np
import ml_dtypes

import concourse.bass as bass
import concourse.mybir as mybir
from concourse import bacc
from concourse.tile import TileContext
from concourse.bass_utils import run_bass_kernel_spmd

BF = mybir.dt.bfloat16
F32 = mybir.dt.float32
F32R = mybir.dt.float32r
AT = mybir.ActivationFunctionType
AL = mybir.AluOpType
bf16 = ml_dtypes.bfloat16

# Problem geometry (hardcoded per spec)
N_IMG, INS_PER_IMG, C, H, W = 4, 8, 64, 160, 256
CIN = C + 2
L = H * W                       # 40960 positions
L1, L2_, L3_ = (C + 2) * C, C * C, C
B1OFF = L1 + L2_ + L3_          # offsets into the 8513-param vector
MASK_BIAS_SHIFT = 2.19

N_CORES = 8
INST_PER_CORE = 4               # 2 pairs
T = 1024                        # positions per chunk
NCHUNK = L // T                 # 40
GROUPS = NCHUNK * 4             # 160 [2, 512] position-groups per core
N_BANKS = (GROUPS + 63) // 64   # 3 output PSUM bank fills (64, 64, 32 groups)

# relu op cost estimates (ns) for greedy ACT/DVE balancing
COST_DVE = (120 + T) / 0.96
COST_ACT = (352 + T) / 1.2

_cache = {}


def _build_program():
    nc = bacc.Bacc("TRN2", target_bir_lowering=False, debug=False)

    xp = nc.dram_tensor("xp", [CIN + 1, L], F32R, kind="ExternalInput")
    w0t = [nc.dram_tensor(f"w0t{p}", [CIN + 1, 128], F32R, kind="ExternalInput")
           for p in range(2)]
    w1t = [nc.dram_tensor(f"w1t{p}", [128, 128], BF, kind="ExternalInput")
           for p in range(2)]
    w2p = [nc.dram_tensor(f"w2p{p}", [128, 512], BF, kind="ExternalInput")
           for p in range(2)]
    b1v = [nc.dram_tensor(f"b1v{p}", [128, 1], F32, kind="ExternalInput")
           for p in range(2)]
    b2v = nc.dram_tensor("b2v", [128, 1], F32, kind="ExternalInput")
    o = nc.dram_tensor("o", [INST_PER_CORE, L], F32, kind="ExternalOutput")

    eng_ns = {"dve": 0.0, "act": 0.0}

    def relu(dst, src, bias_ap):
        if bias_ap is None and eng_ns["dve"] + COST_DVE <= eng_ns["act"] + COST_ACT:
            eng_ns["dve"] += COST_DVE
            if bias_ap is None:
                nc.vector.tensor_scalar(out=dst, in0=src, scalar1=0.0,
                                        scalar2=None, op0=AL.max)
            else:
                nc.vector.tensor_scalar(out=dst, in0=src, scalar1=bias_ap,
                                        scalar2=0.0, op0=AL.add, op1=AL.max)
        else:
            eng_ns["act"] += COST_ACT
            if bias_ap is None:
                nc.scalar.activation(dst, src, AT.Relu)
            else:
                nc.scalar.activation(dst, src, AT.Relu, bias=bias_ap)

    with TileContext(nc) as tc:
        with tc.tile_pool(name="consts", bufs=1) as cpool, \
             tc.tile_pool(name="xpool", bufs=3) as xpool, \
             tc.tile_pool(name="zpool", bufs=3, space="PSUM") as zpool, \
             tc.tile_pool(name="opool", bufs=2, space="PSUM") as opool, \
             tc.tile_pool(name="h1pool", bufs=3) as h1pool, \
             tc.tile_pool(name="h2pool", bufs=3) as h2pool, \
             tc.tile_pool(name="spool", bufs=2) as spool:

            w0_sb, w1_sb, w2_sb, b1_sb = [], [], [], []
            for p in range(2):
                t0 = cpool.tile([CIN + 1, 128], F32R, name=f"w0sb{p}")
                nc.sync.dma_start(out=t0, in_=w0t[p].ap())
                w0_sb.append(t0)
                t1 = cpool.tile([128, 128], BF, name=f"w1sb{p}")
                nc.sync.dma_start(out=t1, in_=w1t[p].ap())
                w1_sb.append(t1)
                t2 = cpool.tile([128, 512], BF, name=f"w2sb{p}")
                nc.sync.dma_start(out=t2, in_=w2p[p].ap())
                w2_sb.append(t2)
                t3 = cpool.tile([128, 1], F32, name=f"b1sb{p}")
                nc.sync.dma_start(out=t3, in_=b1v[p].ap())
                b1_sb.append(t3)
            b2_sb = cpool.tile([128, 1], F32, name="b2sb")
            nc.sync.dma_start(out=b2_sb, in_=b2v.ap())

            feats = {}   # chunk -> tile
            z1s, z2s, h1s, h2s = {}, {}, {}, {}
            obank = {"tile": None, "idx": -1}

            def flush_obank(nrows):
                ob = obank["tile"]
                b = obank["idx"]
                stage = spool.tile([128, 512], F32, name=f"stage{b}", tag="stage")
                nc.vector.tensor_scalar(out=stage[:nrows], in0=ob[:nrows],
                                        scalar1=b2_sb[:nrows, 0:1], scalar2=None,
                                        op0=AL.add)
                ncb = nrows // 8
                # partition q = 8*cb + 4*pair + 2*s + m ; DRAM offset =
                # (2*pair+m)*L + (16*b+cb)*1024 + s*512. One DMA per (pair, s)
                # keeps both APs at <=3 dims.
                src5 = stage.rearrange("(cb pr ss m) f -> cb pr ss m f",
                                       pr=2, ss=2, m=2)
                for pair in range(2):
                    for s in range(2):
                        for mm in range(2):
                            dst = bass.AP(o,
                                          b * 16 * T + (2 * pair + mm) * L + s * 512,
                                          [[T, ncb], [1, 512]])
                            nc.sync.dma_start(out=dst,
                                              in_=src5[:ncb, pair, s, mm, :])

            # software-pipelined emission: iter i does DMA(i+1), L1(i),
            # L3(i-2), L2(i-1); relus follow their producers.
            for i in range(NCHUNK + 2):
                if i == 0:
                    f0 = xpool.tile([CIN + 1, T], F32R, name="feats0", tag="feats")
                    nc.sync.dma_start(out=f0, in_=xp.ap()[:, 0:T])
                    feats[0] = f0
                if i + 1 < NCHUNK:
                    fn = xpool.tile([CIN + 1, T], F32R, name=f"feats{i+1}",
                                    tag="feats")
                    nc.sync.dma_start(out=fn, in_=xp.ap()[:, (i + 1) * T:(i + 2) * T])
                    feats[i + 1] = fn

                # L1(i)
                if i < NCHUNK:
                    for p in range(2):
                        z1 = zpool.tile([128, T], F32, name=f"z1_{i}_{p}", tag="z")
                        for s in range(2):
                            nc.tensor.matmul(z1[:, s * 512:(s + 1) * 512], w0_sb[p],
                                             feats[i][:, s * 512:(s + 1) * 512],
                                             start=True, stop=True)
                        z1s[(i, p)] = z1
                    for p in range(2):
                        h1 = h1pool.tile([128, T], BF, name=f"h1_{i}_{p}", tag="h1")
                        relu(h1, z1s.pop((i, p)), None)
                        h1s[(i, p)] = h1
                    feats.pop(i)

                # L3(i-2)
                j3 = i - 2
                if j3 >= 0:
                    for p in range(2):
                        h2 = h2s[(j3, p)]
                        for s in range(2):
                            g = j3 * 4 + p * 2 + s
                            lg = g % 64
                            if lg == 0:
                                obank["tile"] = opool.tile([128, 512], F32,
                                                           name=f"ob{g//64}",
                                                           tag="ob")
                                obank["idx"] = g // 64
                            jj, jv = lg // 16, lg % 16
                            nc.tensor.matmul(
                                obank["tile"][32 * jj:32 * jj + 32, :],
                                w2_sb[p][:, 32 * jv:32 * jv + 32],
                                h2[:, s * 512:(s + 1) * 512],
                                start=(jv == 0), stop=(jv == 15),
                                tile_position=(0, 32 * jj))
                            if g == GROUPS - 1:
                                flush_obank(((g % 64) + 1) * 2)
                            elif lg == 63:
                                flush_obank(128)
                        if j3 >= 1:
                            h2s.pop((j3 - 1, p), None)

                # L2(i-1)
                j2 = i - 1
                if 0 <= j2 < NCHUNK:
                    for p in range(2):
                        z2 = zpool.tile([128, T], F32, name=f"z2_{j2}_{p}", tag="z")
                        h1 = h1s.pop((j2, p))
                        for s in range(2):
                            nc.tensor.matmul(z2[:, s * 512:(s + 1) * 512], w1_sb[p],
                                             h1[:, s * 512:(s + 1) * 512],
                                             start=True, stop=True)
                        z2s[(j2, p)] = z2
                    for p in range(2):
                        h2 = h2pool.tile([128, T], BF, name=f"h2_{j2}_{p}", tag="h2")
                        relu(h2, z2s.pop((j2, p)), b1_sb[p][:, 0:1])
                        h2s[(j2, p)] = h2

    nc.compile()
    return nc


def _prep_inputs(x, mask_head_params, num_ins):
    x = np.asarray(x, dtype=np.float32)
    params = np.asarray(mask_head_params, dtype=np.float32)
    num_ins = np.asarray(num_ins)
    img_idx = np.repeat(np.arange(N_IMG), num_ins)
    assert img_idx.shape[0] == N_IMG * INS_PER_IMG

    # xplus per image: [locx; locy; ones; x]  -> [67, L] fp32
    loc_x = np.broadcast_to(np.arange(W, dtype=np.float32)[None, :], (H, W))
    loc_y = np.broadcast_to(np.arange(H, dtype=np.float32)[:, None], (H, W))
    xplus = np.empty((N_IMG, CIN + 1, L), dtype=np.float32)
    for img in range(N_IMG):
        xplus[img, 0] = loc_x.reshape(L)
        xplus[img, 1] = loc_y.reshape(L)
        xplus[img, 2] = 1.0
        xplus[img, 3:] = x[img].reshape(C, L)

    in_maps = []
    for c in range(N_CORES):
        inst = [4 * c + k for k in range(INST_PER_CORE)]
        imgs = {img_idx[q] for q in inst}
        assert len(imgs) == 1, "expected each core's instances on one image"
        m = {"xp": xplus[img_idx[inst[0]]]}
        for p in range(2):
            a, b = inst[2 * p], inst[2 * p + 1]
            w0_a = params[a, :L1].reshape(C, CIN)
            w0_b = params[b, :L1].reshape(C, CIN)
            b0_a = params[a, B1OFF:B1OFF + C]
            b0_b = params[b, B1OFF:B1OFF + C]
            # lhsT rows: [w0[:,0]; w0[:,1]; b0; w0[:,2:].T] per instance col blk
            w0t = np.zeros((CIN + 1, 128), np.float32)
            for k, (wv, bv) in enumerate(((w0_a, b0_a), (w0_b, b0_b))):
                cols = slice(64 * k, 64 * k + 64)
                w0t[0, cols] = wv[:, 0]
                w0t[1, cols] = wv[:, 1]
                w0t[2, cols] = bv
                w0t[3:, cols] = wv[:, 2:].T
            m[f"w0t{p}"] = w0t

            w1_a = params[a, L1:L1 + L2_].reshape(C, C)
            w1_b = params[b, L1:L1 + L2_].reshape(C, C)
            w1t = np.zeros((128, 128), np.float32)
            w1t[:64, :64] = w1_a.T
            w1t[64:, 64:] = w1_b.T
            m[f"w1t{p}"] = w1t.astype(bf16)

            w2_a = params[a, L1 + L2_:L1 + L2_ + C]
            w2_b = params[b, L1 + L2_:L1 + L2_ + C]
            w2pair = np.zeros((128, 2), np.float32)
            w2pair[:64, 0] = w2_a
            w2pair[64:, 1] = w2_b
            w2pad = np.zeros((128, 512), np.float32)
            for j in range(16):
                w2pad[:, 32 * j + 2 * j:32 * j + 2 * j + 2] = w2pair
            m[f"w2p{p}"] = w2pad.astype(bf16)

            b1 = np.concatenate([params[a, B1OFF + C:B1OFF + 2 * C],
                                 params[b, B1OFF + C:B1OFF + 2 * C]])
            m[f"b1v{p}"] = b1.reshape(128, 1).astype(np.float32)

        # b2 per out-bank partition q: pair=((q//2)%4)//2, inst_in_pair=q%2
        b2 = np.empty((128, 1), np.float32)
        for q in range(128):
            pair = ((q // 2) % 4) // 2
            mm = q % 2
            iid = inst[2 * pair + mm]
            b2[q, 0] = params[iid, B1OFF + 2 * C] - MASK_BIAS_SHIFT
        m["b2v"] = b2
        in_maps.append(m)
    return in_maps


def kernel(x, mask_head_params, num_ins):
    if "nc" not in _cache:
        _cache["nc"] = _build_program()
    nc = _cache["nc"]
    in_maps = _prep_inputs(x, mask_head_params, num_ins)
    res = run_bass_kernel_spmd(nc, in_maps, core_ids=list(range(N_CORES)))
    out = np.concatenate([r["o"] for r in res.results], axis=0)  # [32, L]
    return out.reshape(1, N_IMG * INS_PER_IMG, H, W).astype(np.float32)



# revision 49
# speedup vs baseline: 12063.8292x; 12063.8292x over previous
"""CondLaneHead DynamicMaskHead kernel for 8 Trainium2 NeuronCores.

Problem: per-instance 3-layer 1x1-conv MLP over a [64,160,256] feature map.
  feats = concat([loc_x, loc_y], x[img])            # [66, L], L = 160*256
  h1 = relu(w0 @ feats + b0)                        # [64, L]
  h2 = relu(w1 @ h1 + b1)                           # [64, L]
  out = w2 @ h2 + b2 - 2.19                         # [1, L]
32 instances (8 per image, 4 images). Sharding: 4 instances per core; each
core needs exactly one image's feature map.

Device mapping (per core, 4 instances = 2 pairs):
  - Layer 1 per pair is a pair of row-tiled matmuls that run CONCURRENTLY on
    the PE (disjoint 32-row groups) and accumulate into one PSUM tile:
      rows 0:64  : x part, fp8 e4m3 (weights and activations)
      rows 64:67 : [loc_x; loc_y; ones] affine part in bf16 (coords must
                   stay bf16: integers up to 255 are exact there)
    fp8 on the x rows halves the dominant HBM/DMA stream; its error lands
    in the coord-dominated h1 at ~0.1% relative.
  - Layer 2: block-diagonal [128, 128] bf16 weights, one matmul per pair.
  - Layer 3 (64->1): output packed across PSUM partitions. Matmuls write
    [32, 512] windows at partition bases 0/32/64/96 using zero-padded weight
    variants (w2 placed at columns 2j of window j), accumulating over 16
    position-groups per window; each completed window is bias-added and
    scatter-DMA'd immediately (fine-grained flush keeps the tail short).
  - DMA plan (the v1-v5 bottleneck): bulk feats ride the gpsimd SWDGE path
    (only path that sprays descriptors across all 16 SDMA engines,
    ~52-72 GB/s; the HWDGE rings drain at ~8 GB/s on one engine). Q7
    descriptor generation is ~70ns/descriptor and serializes, so the fp8
    x-weights ride as a 256-col header inside the first x-super (both are
    64-partition fp8) and the coord weights as a 256-col header inside the
    single coords DMA (3 descriptors for the whole image). Output windows
    flush in bf16 on the otherwise-idle sync HWDGE ring.
"""

import sys

if "/opt/trn_rl_repo" not in sys.path:
    sys.path.insert(0, "/opt/trn_rl_repo")

import numpy as np
import ml_dtypes

import concourse.bass as bass
import concourse.mybir as mybir
from concourse import bacc
from concourse.tile import TileContext
from concourse.bass_utils import run_bass_kernel_spmd

BF = mybir.dt.bfloat16
F32 = mybir.dt.float32
F8 = mybir.dt.float8e4
AT = mybir.ActivationFunctionType
AL = mybir.AluOpType
bf16 = ml_dtypes.bfloat16
f8e4 = ml_dtypes.float8_e4m3

# Problem geometry (hardcoded per spec)
N_IMG, INS_PER_IMG, C, H, W = 4, 8, 64, 160, 256
CIN = C + 2
L = H * W                       # 40960 positions
L1, L2_, L3_ = (C + 2) * C, C * C, C
B1OFF = L1 + L2_ + L3_          # offsets into the 8513-param vector
MASK_BIAS_SHIFT = 2.19

N_CORES = 8
INST_PER_CORE = 4               # 2 pairs
T = 1024                        # positions per chunk
NCHUNK = L // T                 # 40
WHDR = 256                      # weight-header cols in the x8/coords streams
GROUPS = NCHUNK * 4             # 160 [2, 512] position-groups per core
# graded x8 supers (chunk ranges); s0 also carries the fp8 weight header
SUPERS = [(0, 4), (4, 8), (8, 16), (16, 24), (24, 32), (32, 40)]
# super s emitted at END of loop iteration c0(s)-1 (latest point where no
# earlier chunk's compute is emitted after it -- any compute emitted after
# a SWDGE prep waits for that DMA's data)
SUPER_EMIT = {3: 1, 7: 2, 15: 3, 23: 4, 31: 5}

# relu op cost estimates (ns) for greedy ACT/DVE balancing
COST_DVE = (120 + T) / 0.96
COST_ACT = (352 + T) / 1.2

_cache = {}


def _build_program():
    nc = bacc.Bacc("TRN2", target_bir_lowering=False, debug=False)

    # x rows in fp8 e4m3, with the per-pair L1 x-weights as a 256-col header
    x8 = nc.dram_tensor("x8", [C, WHDR + L], F8, kind="ExternalInput")
    # [loc_x; loc_y; ones] rows in bf16, with the per-pair coord-weight
    # lhsT ([w0[:,0]; w0[:,1]; b0] x 128 outs) as a 256-col header
    xc = nc.dram_tensor("xc", [3, WHDR + L], BF, kind="ExternalInput")
    # packed constants: [0:512] w2p0 | [512:1024] w2p1 | [1024:1152] w1t0 |
    # [1152:1280] w1t1 | [1280:1286] f32 biases as bf16 (hi, lo) pairs:
    # b1p0, b1p1, b2
    wpk = nc.dram_tensor("wpk", [128, 1288], BF, kind="ExternalInput")
    o = nc.dram_tensor("o", [INST_PER_CORE, L], BF, kind="ExternalOutput")

    eng_ns = {"dve": 0.0, "act": 0.0}

    def relu(dst, src, bias_ap):
        if eng_ns["dve"] + COST_DVE <= eng_ns["act"] + COST_ACT:
            eng_ns["dve"] += COST_DVE
            if bias_ap is None:
                nc.vector.tensor_scalar(out=dst, in0=src, scalar1=0.0,
                                        scalar2=None, op0=AL.max)
            else:
                nc.vector.tensor_scalar(out=dst, in0=src, scalar1=bias_ap,
                                        scalar2=0.0, op0=AL.add, op1=AL.max)
        else:
            eng_ns["act"] += COST_ACT
            if bias_ap is None:
                nc.scalar.activation(dst, src, AT.Relu)
            else:
                nc.scalar.activation(dst, src, AT.Relu, bias=bias_ap)

    with TileContext(nc) as tc:
        with tc.tile_pool(name="consts", bufs=1) as cpool, \
             tc.tile_pool(name="xpool", bufs=3) as xpool, \
             tc.tile_pool(name="zpool", bufs=3, space="PSUM") as zpool, \
             tc.tile_pool(name="opool", bufs=2, space="PSUM") as opool, \
             tc.tile_pool(name="h1pool", bufs=6) as h1pool, \
             tc.tile_pool(name="h2pool", bufs=6) as h2pool, \
             tc.tile_pool(name="spool", bufs=4) as spool:

            # ---- constant + feats streams, all on the gpsimd SWDGE path,
            # emitted just-in-time in consumption order: compute emitted
            # after a DMA empirically waits for ALL prior SWDGE DMAs, so
            # nothing may be emitted earlier than needed ----
            feat_tiles = {}   # chunk -> (tile, col offset of chunk start)

            def fetch_super(s):
                # each super tile is a [67, range] bf16 tile assembled by two
                # SWDGE DMAs: the fp8 x rows cast in-flight to bf16 into
                # partitions 0:64, the bf16 coord rows into partitions 64:67
                c0, c1 = SUPERS[s]
                if s == 0:
                    t = cpool.tile([CIN + 1, WHDR + 4 * T], BF, name="s0t")
                    lo, hi = 0, WHDR + 4 * T
                else:
                    t = xpool.tile([CIN + 1, 8 * T], BF, name=f"sup{s}",
                                   tag="sup")
                    lo, hi = WHDR + c0 * T, WHDR + c1 * T
                nc.gpsimd.dma_start(out=t[0:C, 0:hi - lo],
                                    in_=x8.ap()[:, lo:hi])
                nc.gpsimd.dma_start(out=t[C:CIN + 1, 0:hi - lo],
                                    in_=xc.ap()[:, lo:hi])
                for k in range(c0, c1):
                    feat_tiles[k] = (t, (WHDR if s == 0 else 0) + (k - c0) * T)
                return t

            s0t = fetch_super(0)
            w0_sb = [s0t[:, 128 * p:128 * (p + 1)] for p in range(2)]

            wsb = cpool.tile([128, 1288], BF, name="wsb")
            fsb = cpool.tile([128, 3], F32, name="fsb")
            w2_sb = [wsb[:, 512 * p:512 * (p + 1)] for p in range(2)]
            w1_sb = [wsb[:, 1024 + 128 * p:1024 + 128 * (p + 1)]
                     for p in range(2)]
            b1_sb = [fsb[:, p:p + 1] for p in range(2)]

            def load_wpk():
                nc.gpsimd.dma_start(out=wsb, in_=wpk.ap())
                for k in range(3):
                    nc.vector.tensor_tensor(
                        out=fsb[:, k:k + 1],
                        in0=wsb[:, 1280 + 2 * k:1281 + 2 * k],
                        in1=wsb[:, 1281 + 2 * k:1282 + 2 * k],
                        op=AL.add)

            z1s, z2s, h1s, h2s = {}, {}, {}, {}
            obank = {"tile": None, "idx": -1}

            def flush_window(g):
                # window jj of bank b just completed (16 groups x 2 rows)
                b, jj = g // 64, (g % 64) // 16
                ob = obank["tile"]
                stage = spool.tile([32, 512], BF, name=f"st{g//16}", tag="st")
                nc.vector.tensor_scalar(out=stage,
                                        in0=ob[32 * jj:32 * jj + 32, :],
                                        scalar1=fsb[32 * jj:32 * jj + 32, 2:3],
                                        scalar2=None, op0=AL.add)
                eng_ns["dve"] += (120 + 512) / 0.96
                # stage partition 8cb+4p+2s+m <-> instance 2p+m, position
                # (16b+4jj+cb)*T + s*512; one DMA per (p, s, m) keeps APs 2-D
                src = stage.rearrange("(cb p s m) f -> cb p s m f",
                                      p=2, s=2, m=2)
                # the final window flushes via gpsimd SWDGE (fast, and no
                # compute is emitted after it); the rest ride the sync ring
                eng = nc.gpsimd if g == GROUPS - 1 else nc.sync
                for par in range(2):
                    for s in range(2):
                        for m in range(2):
                            dst = bass.AP(
                                o,
                                (2 * par + m) * L + (16 * b + 4 * jj) * T + s * 512,
                                [[T, 4], [1, 512]])
                            eng.dma_start(out=dst, in_=src[:, par, s, m, :])

            # software-pipelined emission: iter i does L1(i), L3(i-2),
            # L2(i-1); relus follow their producers. DMAs are emitted at
            # the END of an iteration (after that L1) so earlier compute
            # never gates on them.
            for i in range(NCHUNK + 2):
                # L1(i): one [67,128] x [67,512] matmul per (pair, half);
                # feats rows 0:64 = x (cast from fp8), 64:67 = coords
                if i < NCHUNK:
                    ft, off = feat_tiles[i]
                    for p in range(2):
                        z1 = zpool.tile([128, T], F32, name=f"z1_{i}_{p}", tag="z")
                        for s in range(2):
                            nc.tensor.matmul(
                                z1[:, s * 512:(s + 1) * 512], w0_sb[p],
                                ft[:, off + s * 512:off + (s + 1) * 512],
                                start=True, stop=True)
                        z1s[(i, p)] = z1
                    for p in range(2):
                        h1 = h1pool.tile([128, T], BF, name=f"h1_{i}_{p}", tag="h1")
                        relu(h1, z1s.pop((i, p)), None)
                        h1s[(i, p)] = h1
                    feat_tiles.pop(i)

                if i == 0:
                    load_wpk()
                elif i in SUPER_EMIT:
                    fetch_super(SUPER_EMIT[i])

                # L3(i-2)
                j3 = i - 2
                if j3 >= 0:
                    for p in range(2):
                        h2 = h2s[(j3, p)]
                        for s in range(2):
                            g = j3 * 4 + p * 2 + s
                            lg = g % 64
                            if lg == 0:
                                obank["tile"] = opool.tile([128, 512], F32,
                                                           name=f"ob{g//64}",
                                                           tag="ob")
                                obank["idx"] = g // 64
                            jj, jv = lg // 16, lg % 16
                            nc.tensor.matmul(
                                obank["tile"][32 * jj:32 * jj + 32, :],
                                w2_sb[p][:, 32 * jv:32 * jv + 32],
                                h2[:, s * 512:(s + 1) * 512],
                                start=(jv == 0), stop=(jv == 15),
                                tile_position=(0, 32 * jj))
                            if jv == 15:
                                flush_window(g)
                        if j3 >= 1:
                            h2s.pop((j3 - 1, p), None)

                # L2(i-1)
                j2 = i - 1
                if 0 <= j2 < NCHUNK:
                    for p in range(2):
                        z2 = zpool.tile([128, T], F32, name=f"z2_{j2}_{p}", tag="z")
                        h1 = h1s.pop((j2, p))
                        for s in range(2):
                            nc.tensor.matmul(z2[:, s * 512:(s + 1) * 512], w1_sb[p],
                                             h1[:, s * 512:(s + 1) * 512],
                                             start=True, stop=True)
                        z2s[(j2, p)] = z2
                    for p in range(2):
                        h2 = h2pool.tile([128, T], BF, name=f"h2_{j2}_{p}", tag="h2")
                        relu(h2, z2s.pop((j2, p)), b1_sb[p])
                        h2s[(j2, p)] = h2

    nc.compile()
    return nc


def _prep_inputs(x, mask_head_params, num_ins):
    x = np.asarray(x, dtype=np.float32)
    params = np.asarray(mask_head_params, dtype=np.float32)
    num_ins = np.asarray(num_ins)
    img_idx = np.repeat(np.arange(N_IMG), num_ins)
    assert img_idx.shape[0] == N_IMG * INS_PER_IMG

    loc_x = np.broadcast_to(np.arange(W, dtype=np.float32)[None, :], (H, W))
    loc_y = np.broadcast_to(np.arange(H, dtype=np.float32)[:, None], (H, W))
    coords = np.stack([loc_x.reshape(L), loc_y.reshape(L),
                       np.ones(L, np.float32)])          # [3, L]

    in_maps = []
    for c in range(N_CORES):
        inst = [4 * c + k for k in range(INST_PER_CORE)]
        imgs = {img_idx[q] for q in inst}
        assert len(imgs) == 1, "expected each core's instances on one image"
        img = img_idx[inst[0]]

        x8 = np.zeros((C, WHDR + L), dtype=f8e4)
        x8[:, WHDR:] = x[img].reshape(C, L).astype(f8e4)
        xc = np.zeros((3, WHDR + L), dtype=bf16)
        xc[0, WHDR:] = coords[0].astype(bf16)
        xc[1, WHDR:] = coords[1].astype(bf16)
        xc[2, WHDR:] = 1.0
        wpk = np.zeros((128, 1288), np.float32)
        fpk = np.zeros((128, 3), np.float32)  # f32 biases: b1p0 | b1p1 | b2

        for p in range(2):
            a, b = inst[2 * p], inst[2 * p + 1]
            w0_a = params[a, :L1].reshape(C, CIN)
            w0_b = params[b, :L1].reshape(C, CIN)
            b0_a = params[a, B1OFF:B1OFF + C]
            b0_b = params[b, B1OFF:B1OFF + C]
            for k, (wv, bv) in enumerate(((w0_a, b0_a), (w0_b, b0_b))):
                cols = slice(128 * p + 64 * k, 128 * p + 64 * k + 64)
                # coord lhsT rows: [w0[:,0]; w0[:,1]; b0]; x lhsT in fp8
                xc[0, cols] = wv[:, 0].astype(bf16)
                xc[1, cols] = wv[:, 1].astype(bf16)
                xc[2, cols] = bv.astype(bf16)
                x8[:, cols] = wv[:, 2:].T.astype(f8e4)

            w1_a = params[a, L1:L1 + L2_].reshape(C, C)
            w1_b = params[b, L1:L1 + L2_].reshape(C, C)
            w1tt = np.zeros((128, 128), np.float32)
            w1tt[:64, :64] = w1_a.T
            w1tt[64:, 64:] = w1_b.T
            wpk[:, 1024 + 128 * p:1024 + 128 * (p + 1)] = w1tt

            w2_a = params[a, L1 + L2_:L1 + L2_ + C]
            w2_b = params[b, L1 + L2_:L1 + L2_ + C]
            w2pair = np.zeros((128, 2), np.float32)
            w2pair[:64, 0] = w2_a
            w2pair[64:, 1] = w2_b
            w2pad = np.zeros((128, 512), np.float32)
            for j in range(16):
                w2pad[:, 32 * j + 2 * j:32 * j + 2 * j + 2] = w2pair
            wpk[:, 512 * p:512 * (p + 1)] = w2pad

            b1 = np.concatenate([params[a, B1OFF + C:B1OFF + 2 * C],
                                 params[b, B1OFF + C:B1OFF + 2 * C]])
            fpk[:, p] = b1

        # b2 per out-bank partition q: pair=((q//2)%4)//2, inst_in_pair=q%2
        for q in range(128):
            pair = ((q // 2) % 4) // 2
            mm = q % 2
            iid = inst[2 * pair + mm]
            fpk[q, 2] = params[iid, B1OFF + 2 * C] - MASK_BIAS_SHIFT
        # biases as bf16 (hi, lo) pairs appended to the weight pack
        hi = fpk.astype(bf16)
        lo = (fpk - hi.astype(np.float32)).astype(bf16)
        pk = wpk.astype(bf16)
        pk[:, 1280:1286:2] = hi
        pk[:, 1281:1287:2] = lo
        in_maps.append({"x8": x8, "xc": xc, "wpk": pk})
    return in_maps


def kernel(x, mask_head_params, num_ins):
    if "nc" not in _cache:
        _cache["nc"] = _build_program()
    nc = _cache["nc"]
    in_maps = _prep_inputs(x, mask_head_params, num_ins)
    res = run_bass_kernel_spmd(nc, in_maps, core_ids=list(range(N_CORES)))
    out = np.concatenate([r["o"].astype(np.float32) for r in res.results], axis=0)
    return out.reshape(1, N_IMG * INS_PER_IMG, H, W)


# revision 50
# speedup vs baseline: 12671.8708x; 1.0504x over previous
"""CondLaneHead DynamicMaskHead kernel for 8 Trainium2 NeuronCores.

Problem: per-instance 3-layer 1x1-conv MLP over a [64,160,256] feature map.
  feats = concat([loc_x, loc_y], x[img])            # [66, L], L = 160*256
  h1 = relu(w0 @ feats + b0)                        # [64, L]
  h2 = relu(w1 @ h1 + b1)                           # [64, L]
  out = w2 @ h2 + b2 - 2.19                         # [1, L]
32 instances (8 per image, 4 images). Sharding: 4 instances per core; each
core needs exactly one image's feature map.

Device mapping (per core, 4 instances = 2 pairs):
  - Layer 1 per pair is a pair of row-tiled matmuls that run CONCURRENTLY on
    the PE (disjoint 32-row groups) and accumulate into one PSUM tile:
      rows 0:64  : x part, fp8 e4m3 (weights and activations)
      rows 64:67 : [loc_x; loc_y; ones] affine part in bf16 (coords must
                   stay bf16: integers up to 255 are exact there)
    fp8 on the x rows halves the dominant HBM/DMA stream; its error lands
    in the coord-dominated h1 at ~0.1% relative.
  - Layer 2: block-diagonal [128, 128] bf16 weights, one matmul per pair.
  - Layer 3 (64->1): output packed across PSUM partitions. Matmuls write
    [32, 512] windows at partition bases 0/32/64/96 using zero-padded weight
    variants (w2 placed at columns 2j of window j), accumulating over 16
    position-groups per window; each completed window is bias-added and
    scatter-DMA'd immediately (fine-grained flush keeps the tail short).
  - DMA plan (the v1-v5 bottleneck): bulk feats ride the gpsimd SWDGE path
    (only path that sprays descriptors across all 16 SDMA engines,
    ~52-72 GB/s; the HWDGE rings drain at ~8 GB/s on one engine). Q7
    descriptor generation is ~70ns/descriptor and serializes, so the fp8
    x-weights ride as a 256-col header inside the first x-super (both are
    64-partition fp8) and the coord weights as a 256-col header inside the
    single coords DMA (3 descriptors for the whole image). Output windows
    flush in bf16 on the otherwise-idle sync HWDGE ring.
"""

import sys

if "/opt/trn_rl_repo" not in sys.path:
    sys.path.insert(0, "/opt/trn_rl_repo")

import numpy as np
import ml_dtypes

import concourse.bass as bass
import concourse.mybir as mybir
from concourse import bacc
from concourse.tile import TileContext
from concourse.bass_utils import run_bass_kernel_spmd

BF = mybir.dt.bfloat16
F32 = mybir.dt.float32
F8 = mybir.dt.float8e4
AT = mybir.ActivationFunctionType
AL = mybir.AluOpType
bf16 = ml_dtypes.bfloat16
f8e4 = ml_dtypes.float8_e4m3

# Problem geometry (hardcoded per spec)
N_IMG, INS_PER_IMG, C, H, W = 4, 8, 64, 160, 256
CIN = C + 2
L = H * W                       # 40960 positions
L1, L2_, L3_ = (C + 2) * C, C * C, C
B1OFF = L1 + L2_ + L3_          # offsets into the 8513-param vector
MASK_BIAS_SHIFT = 2.19

N_CORES = 8
INST_PER_CORE = 4               # 2 pairs
T = 1024                        # positions per chunk
NCHUNK = L // T                 # 40
WHDR = 256                      # weight-header cols in the x8/coords streams
GROUPS = NCHUNK * 4             # 160 [2, 512] position-groups per core
# graded x8 supers (chunk ranges); s0 also carries the fp8 weight header
SUPERS = [(0, 4), (4, 8), (8, 16), (16, 24), (24, 32), (32, 40)]
# super s emitted at END of loop iteration c0(s)-1 (latest point where no
# earlier chunk's compute is emitted after it -- any compute emitted after
# a SWDGE prep waits for that DMA's data)
SUPER_EMIT = {3: 1, 7: 2, 15: 3, 23: 4, 31: 5}

# relu op cost estimates (ns) for greedy ACT/DVE balancing
COST_DVE = (120 + T) / 0.96
COST_ACT = (352 + T) / 1.2

_cache = {}


def _build_program():
    nc = bacc.Bacc("TRN2", target_bir_lowering=False, debug=False)

    # x rows in fp8 e4m3, with the per-pair L1 x-weights as a 256-col header
    x8 = nc.dram_tensor("x8", [C, WHDR + L], F8, kind="ExternalInput")
    # [loc_x; loc_y; ones] rows in bf16, with the per-pair coord-weight
    # lhsT ([w0[:,0]; w0[:,1]; b0] x 128 outs) as a 256-col header
    xc = nc.dram_tensor("xc", [3, WHDR + L], BF, kind="ExternalInput")
    # packed constants: [0:512] w2p0 | [512:1024] w2p1 | [1024:1152] w1t0 |
    # [1152:1280] w1t1 | [1280:1286] f32 biases as bf16 (hi, lo) pairs:
    # b1p0, b1p1, b2
    wpk = nc.dram_tensor("wpk", [128, 1288], BF, kind="ExternalInput")
    o = nc.dram_tensor("o", [INST_PER_CORE, L], BF, kind="ExternalOutput")

    eng_ns = {"dve": 0.0, "act": 0.0}

    def relu(dst, src, bias_ap):
        if eng_ns["dve"] + COST_DVE <= eng_ns["act"] + COST_ACT:
            eng_ns["dve"] += COST_DVE
            if bias_ap is None:
                nc.vector.tensor_scalar(out=dst, in0=src, scalar1=0.0,
                                        scalar2=None, op0=AL.max)
            else:
                nc.vector.tensor_scalar(out=dst, in0=src, scalar1=bias_ap,
                                        scalar2=0.0, op0=AL.add, op1=AL.max)
        else:
            eng_ns["act"] += COST_ACT
            if bias_ap is None:
                nc.scalar.activation(dst, src, AT.Relu)
            else:
                nc.scalar.activation(dst, src, AT.Relu, bias=bias_ap)

    with TileContext(nc) as tc:
        with tc.tile_pool(name="consts", bufs=1) as cpool, \
             tc.tile_pool(name="xpool", bufs=3) as xpool, \
             tc.tile_pool(name="zpool", bufs=3, space="PSUM") as zpool, \
             tc.tile_pool(name="opool", bufs=2, space="PSUM") as opool, \
             tc.tile_pool(name="h1pool", bufs=6) as h1pool, \
             tc.tile_pool(name="h2pool", bufs=6) as h2pool, \
             tc.tile_pool(name="spool", bufs=4) as spool:

            # ---- constant + feats streams, all on the gpsimd SWDGE path,
            # emitted just-in-time in consumption order: compute emitted
            # after a DMA empirically waits for ALL prior SWDGE DMAs, so
            # nothing may be emitted earlier than needed ----
            feat_tiles = {}   # chunk -> (tile, col offset of chunk start)

            def fetch_super(s):
                # each super tile is a [67, range] bf16 tile assembled by two
                # SWDGE DMAs: the fp8 x rows cast in-flight to bf16 into
                # partitions 0:64, the bf16 coord rows into partitions 64:67
                c0, c1 = SUPERS[s]
                if s == 0:
                    t = cpool.tile([CIN + 1, WHDR + 4 * T], BF, name="s0t")
                    lo, hi = 0, WHDR + 4 * T
                else:
                    t = xpool.tile([CIN + 1, 8 * T], BF, name=f"sup{s}",
                                   tag="sup")
                    lo, hi = WHDR + c0 * T, WHDR + c1 * T
                nc.gpsimd.dma_start(out=t[0:C, 0:hi - lo],
                                    in_=x8.ap()[:, lo:hi])
                nc.gpsimd.dma_start(out=t[C:CIN + 1, 0:hi - lo],
                                    in_=xc.ap()[:, lo:hi])
                for k in range(c0, c1):
                    feat_tiles[k] = (t, (WHDR if s == 0 else 0) + (k - c0) * T)
                return t

            s0t = fetch_super(0)
            w0_sb = [s0t[:, 128 * p:128 * (p + 1)] for p in range(2)]

            wsb = cpool.tile([128, 1288], BF, name="wsb")
            fsb = cpool.tile([128, 3], F32, name="fsb")
            w2_sb = [wsb[:, 512 * p:512 * (p + 1)] for p in range(2)]
            w1_sb = [wsb[:, 1024 + 128 * p:1024 + 128 * (p + 1)]
                     for p in range(2)]
            b1_sb = [fsb[:, p:p + 1] for p in range(2)]

            def load_wpk():
                nc.gpsimd.dma_start(out=wsb, in_=wpk.ap())
                for k in range(3):
                    nc.vector.tensor_tensor(
                        out=fsb[:, k:k + 1],
                        in0=wsb[:, 1280 + 2 * k:1281 + 2 * k],
                        in1=wsb[:, 1281 + 2 * k:1282 + 2 * k],
                        op=AL.add)

            z1s, z2s, h1s, h2s = {}, {}, {}, {}
            obank = {"tile": None, "idx": -1}

            def flush_window(g):
                # window jj of bank b just completed (16 groups x 2 rows)
                b, jj = g // 64, (g % 64) // 16
                ob = obank["tile"]
                stage = spool.tile([32, 512], BF, name=f"st{g//16}", tag="st")
                nc.vector.tensor_scalar(out=stage,
                                        in0=ob[32 * jj:32 * jj + 32, :],
                                        scalar1=fsb[32 * jj:32 * jj + 32, 2:3],
                                        scalar2=None, op0=AL.add)
                eng_ns["dve"] += (120 + 512) / 0.96
                # stage partition 8cb+4p+2s+m <-> instance 2p+m, position
                # (16b+4jj+cb)*T + s*512; one DMA per (p, s, m) keeps APs 2-D
                src = stage.rearrange("(cb p s m) f -> cb p s m f",
                                      p=2, s=2, m=2)
                # the final window flushes via gpsimd SWDGE (fast, and no
                # compute is emitted after it); the rest ride the sync ring
                eng = nc.gpsimd if g == GROUPS - 1 else nc.sync
                for par in range(2):
                    for s in range(2):
                        for m in range(2):
                            dst = bass.AP(
                                o,
                                (2 * par + m) * L + (16 * b + 4 * jj) * T + s * 512,
                                [[T, 4], [1, 512]])
                            eng.dma_start(out=dst, in_=src[:, par, s, m, :])

            # software-pipelined emission: iter i does L1(i), L3(i-2),
            # L2(i-1); relus follow their producers. DMAs are emitted at
            # the END of an iteration (after that L1) so earlier compute
            # never gates on them.
            for i in range(NCHUNK + 2):
                # L1(i): one [67,128] x [67,512] matmul per (pair, half);
                # feats rows 0:64 = x (cast from fp8), 64:67 = coords
                if i < NCHUNK:
                    ft, off = feat_tiles[i]
                    for p in range(2):
                        z1 = zpool.tile([128, T], F32, name=f"z1_{i}_{p}", tag="z")
                        for s in range(2):
                            nc.tensor.matmul(
                                z1[:, s * 512:(s + 1) * 512], w0_sb[p],
                                ft[:, off + s * 512:off + (s + 1) * 512],
                                start=True, stop=True)
                        z1s[(i, p)] = z1
                    for p in range(2):
                        h1 = h1pool.tile([128, T], BF, name=f"h1_{i}_{p}", tag="h1")
                        relu(h1, z1s.pop((i, p)), None)
                        h1s[(i, p)] = h1
                    feat_tiles.pop(i)

                if i == 0:
                    load_wpk()
                elif i in SUPER_EMIT:
                    fetch_super(SUPER_EMIT[i])

                # L2(i-1) before L3 so its relus (which free the z slots the
                # next L1 reuses, via the 3-slot FIFO rotation) are emitted
                # a full L3 segment earlier
                j2 = i - 1
                if 0 <= j2 < NCHUNK:
                    for p in range(2):
                        z2 = zpool.tile([128, T], F32, name=f"z2_{j2}_{p}", tag="z")
                        h1 = h1s.pop((j2, p))
                        for s in range(2):
                            nc.tensor.matmul(z2[:, s * 512:(s + 1) * 512], w1_sb[p],
                                             h1[:, s * 512:(s + 1) * 512],
                                             start=True, stop=True)
                        z2s[(j2, p)] = z2
                    for p in range(2):
                        h2 = h2pool.tile([128, T], BF, name=f"h2_{j2}_{p}", tag="h2")
                        relu(h2, z2s.pop((j2, p)), b1_sb[p])
                        h2s[(j2, p)] = h2

                # L3(i-2)
                j3 = i - 2
                if j3 >= 0:
                    for p in range(2):
                        h2 = h2s[(j3, p)]
                        for s in range(2):
                            g = j3 * 4 + p * 2 + s
                            lg = g % 64
                            if lg == 0:
                                obank["tile"] = opool.tile([128, 512], F32,
                                                           name=f"ob{g//64}",
                                                           tag="ob")
                                obank["idx"] = g // 64
                            jj, jv = lg // 16, lg % 16
                            nc.tensor.matmul(
                                obank["tile"][32 * jj:32 * jj + 32, :],
                                w2_sb[p][:, 32 * jv:32 * jv + 32],
                                h2[:, s * 512:(s + 1) * 512],
                                start=(jv == 0), stop=(jv == 15),
                                tile_position=(0, 32 * jj))
                            if jv == 15:
                                flush_window(g)
                        if j3 >= 1:
                            h2s.pop((j3 - 1, p), None)

    nc.compile()
    return nc


def _prep_inputs(x, mask_head_params, num_ins):
    x = np.asarray(x, dtype=np.float32)
    params = np.asarray(mask_head_params, dtype=np.float32)
    num_ins = np.asarray(num_ins)
    img_idx = np.repeat(np.arange(N_IMG), num_ins)
    assert img_idx.shape[0] == N_IMG * INS_PER_IMG

    loc_x = np.broadcast_to(np.arange(W, dtype=np.float32)[None, :], (H, W))
    loc_y = np.broadcast_to(np.arange(H, dtype=np.float32)[:, None], (H, W))
    coords = np.stack([loc_x.reshape(L), loc_y.reshape(L),
                       np.ones(L, np.float32)])          # [3, L]

    in_maps = []
    for c in range(N_CORES):
        inst = [4 * c + k for k in range(INST_PER_CORE)]
        imgs = {img_idx[q] for q in inst}
        assert len(imgs) == 1, "expected each core's instances on one image"
        img = img_idx[inst[0]]

        x8 = np.zeros((C, WHDR + L), dtype=f8e4)
        x8[:, WHDR:] = x[img].reshape(C, L).astype(f8e4)
        xc = np.zeros((3, WHDR + L), dtype=bf16)
        xc[0, WHDR:] = coords[0].astype(bf16)
        xc[1, WHDR:] = coords[1].astype(bf16)
        xc[2, WHDR:] = 1.0
        wpk = np.zeros((128, 1288), np.float32)
        fpk = np.zeros((128, 3), np.float32)  # f32 biases: b1p0 | b1p1 | b2

        for p in range(2):
            a, b = inst[2 * p], inst[2 * p + 1]
            w0_a = params[a, :L1].reshape(C, CIN)
            w0_b = params[b, :L1].reshape(C, CIN)
            b0_a = params[a, B1OFF:B1OFF + C]
            b0_b = params[b, B1OFF:B1OFF + C]
            for k, (wv, bv) in enumerate(((w0_a, b0_a), (w0_b, b0_b))):
                cols = slice(128 * p + 64 * k, 128 * p + 64 * k + 64)
                # coord lhsT rows: [w0[:,0]; w0[:,1]; b0]; x lhsT in fp8
                xc[0, cols] = wv[:, 0].astype(bf16)
                xc[1, cols] = wv[:, 1].astype(bf16)
                xc[2, cols] = bv.astype(bf16)
                x8[:, cols] = wv[:, 2:].T.astype(f8e4)

            w1_a = params[a, L1:L1 + L2_].reshape(C, C)
            w1_b = params[b, L1:L1 + L2_].reshape(C, C)
            w1tt = np.zeros((128, 128), np.float32)
            w1tt[:64, :64] = w1_a.T
            w1tt[64:, 64:] = w1_b.T
            wpk[:, 1024 + 128 * p:1024 + 128 * (p + 1)] = w1tt

            w2_a = params[a, L1 + L2_:L1 + L2_ + C]
            w2_b = params[b, L1 + L2_:L1 + L2_ + C]
            w2pair = np.zeros((128, 2), np.float32)
            w2pair[:64, 0] = w2_a
            w2pair[64:, 1] = w2_b
            w2pad = np.zeros((128, 512), np.float32)
            for j in range(16):
                w2pad[:, 32 * j + 2 * j:32 * j + 2 * j + 2] = w2pair
            wpk[:, 512 * p:512 * (p + 1)] = w2pad

            b1 = np.concatenate([params[a, B1OFF + C:B1OFF + 2 * C],
                                 params[b, B1OFF + C:B1OFF + 2 * C]])
            fpk[:, p] = b1

        # b2 per out-bank partition q: pair=((q//2)%4)//2, inst_in_pair=q%2
        for q in range(128):
            pair = ((q // 2) % 4) // 2
            mm = q % 2
            iid = inst[2 * pair + mm]
            fpk[q, 2] = params[iid, B1OFF + 2 * C] - MASK_BIAS_SHIFT
        # biases as bf16 (hi, lo) pairs appended to the weight pack
        hi = fpk.astype(bf16)
        lo = (fpk - hi.astype(np.float32)).astype(bf16)
        pk = wpk.astype(bf16)
        pk[:, 1280:1286:2] = hi
        pk[:, 1281:1287:2] = lo
        in_maps.append({"x8": x8, "xc": xc, "wpk": pk})
    return in_maps


def kernel(x, mask_head_params, num_ins):
    if "nc" not in _cache:
        _cache["nc"] = _build_program()
    nc = _cache["nc"]
    in_maps = _prep_inputs(x, mask_head_params, num_ins)
    res = run_bass_kernel_spmd(nc, in_maps, core_ids=list(range(N_CORES)))
    out = np.concatenate([r["o"].astype(np.float32) for r in res.results], axis=0)
    return out.reshape(1, N_IMG * INS_PER_IMG, H, W)


# revision 54
# speedup vs baseline: 12810.6189x; 1.0109x over previous
"""CondLaneHead DynamicMaskHead kernel for 8 Trainium2 NeuronCores.

Problem: per-instance 3-layer 1x1-conv MLP over a [64,160,256] feature map.
  feats = concat([loc_x, loc_y], x[img])            # [66, L], L = 160*256
  h1 = relu(w0 @ feats + b0)                        # [64, L]
  h2 = relu(w1 @ h1 + b1)                           # [64, L]
  out = w2 @ h2 + b2 - 2.19                         # [1, L]
32 instances (8 per image, 4 images). Sharding: 4 instances per core; each
core needs exactly one image's feature map.

Device mapping (per core, 4 instances = 2 pairs):
  - Layer 1 per pair is a pair of row-tiled matmuls that run CONCURRENTLY on
    the PE (disjoint 32-row groups) and accumulate into one PSUM tile:
      rows 0:64  : x part, fp8 e4m3 (weights and activations)
      rows 64:67 : [loc_x; loc_y; ones] affine part in bf16 (coords must
                   stay bf16: integers up to 255 are exact there)
    fp8 on the x rows halves the dominant HBM/DMA stream; its error lands
    in the coord-dominated h1 at ~0.1% relative.
  - Layer 2: block-diagonal [128, 128] bf16 weights, one matmul per pair.
  - Layer 3 (64->1): output packed across PSUM partitions. Matmuls write
    [32, 512] windows at partition bases 0/32/64/96 using zero-padded weight
    variants (w2 placed at columns 2j of window j), accumulating over 16
    position-groups per window; each completed window is bias-added and
    scatter-DMA'd immediately (fine-grained flush keeps the tail short).
  - DMA plan (the v1-v5 bottleneck): bulk feats ride the gpsimd SWDGE path
    (only path that sprays descriptors across all 16 SDMA engines,
    ~52-72 GB/s; the HWDGE rings drain at ~8 GB/s on one engine). Q7
    descriptor generation is ~70ns/descriptor and serializes, so the fp8
    x-weights ride as a 256-col header inside the first x-super (both are
    64-partition fp8) and the coord weights as a 256-col header inside the
    single coords DMA (3 descriptors for the whole image). Output windows
    flush in bf16 on the otherwise-idle sync HWDGE ring.
"""

import sys

if "/opt/trn_rl_repo" not in sys.path:
    sys.path.insert(0, "/opt/trn_rl_repo")

import numpy as np
import ml_dtypes

import concourse.bass as bass
import concourse.mybir as mybir
from concourse import bacc
from concourse.tile import TileContext
from concourse.bass_utils import run_bass_kernel_spmd

BF = mybir.dt.bfloat16
F32 = mybir.dt.float32
F8 = mybir.dt.float8e4
AT = mybir.ActivationFunctionType
AL = mybir.AluOpType
bf16 = ml_dtypes.bfloat16
f8e4 = ml_dtypes.float8_e4m3

# Problem geometry (hardcoded per spec)
N_IMG, INS_PER_IMG, C, H, W = 4, 8, 64, 160, 256
CIN = C + 2
L = H * W                       # 40960 positions
L1, L2_, L3_ = (C + 2) * C, C * C, C
B1OFF = L1 + L2_ + L3_          # offsets into the 8513-param vector
MASK_BIAS_SHIFT = 2.19

N_CORES = 8
INST_PER_CORE = 4               # 2 pairs
T = 1024                        # positions per chunk
NCHUNK = L // T                 # 40
WHDR = 256                      # weight-header cols in the x8/coords streams
GROUPS = NCHUNK * 4             # 160 [2, 512] position-groups per core
# graded x8 supers (chunk ranges); s0 also carries the fp8 weight header
SUPERS = [(0, 4), (4, 8), (8, 16), (16, 24), (24, 32), (32, 40)]
# super s emitted at END of loop iteration c0(s)-1 (latest point where no
# earlier chunk's compute is emitted after it -- any compute emitted after
# a SWDGE prep waits for that DMA's data)
SUPER_EMIT = {3: 1, 7: 2, 15: 3, 23: 4, 31: 5}

# relu op cost estimates (ns) for greedy ACT/DVE balancing
COST_DVE = (120 + T) / 0.96
COST_ACT = (352 + T) / 1.2

_cache = {}


def _build_program():
    nc = bacc.Bacc("TRN2", target_bir_lowering=False, debug=False)

    # x rows in fp8 e4m3, with the per-pair L1 x-weights as a 256-col header
    x8 = nc.dram_tensor("x8", [C, WHDR + L], F8, kind="ExternalInput")
    # [loc_x; loc_y; ones] rows in bf16, with the per-pair coord-weight
    # lhsT ([w0[:,0]; w0[:,1]; b0] x 128 outs) as a 256-col header
    xc = nc.dram_tensor("xc", [3, WHDR + L], BF, kind="ExternalInput")
    # packed constants: [0:512] w2p0 | [512:1024] w2p1 | [1024:1152] w1t0 |
    # [1152:1280] w1t1 | [1280:1286] f32 biases as bf16 (hi, lo) pairs:
    # b1p0, b1p1, b2
    wpk = nc.dram_tensor("wpk", [128, 1288], BF, kind="ExternalInput")
    o = nc.dram_tensor("o", [INST_PER_CORE, L], BF, kind="ExternalOutput")

    def relu(dst, src, bias_ap, on_act):
        # Engine is chosen by slot criticality, not greedy balance: the
        # 3-slot PSUM FIFO rotation reuses z1p0/z2p0's slots soonest, so
        # those relus go to the lower-latency, less-loaded ACT engine.
        if on_act:
            if bias_ap is None:
                nc.scalar.activation(dst, src, AT.Relu)
            else:
                nc.scalar.activation(dst, src, AT.Relu, bias=bias_ap)
        else:
            if bias_ap is None:
                nc.vector.tensor_scalar(out=dst, in0=src, scalar1=0.0,
                                        scalar2=None, op0=AL.max)
            else:
                nc.vector.tensor_scalar(out=dst, in0=src, scalar1=bias_ap,
                                        scalar2=0.0, op0=AL.add, op1=AL.max)

    with TileContext(nc) as tc:
        with tc.tile_pool(name="consts", bufs=1) as cpool, \
             tc.tile_pool(name="xpool", bufs=3) as xpool, \
             tc.tile_pool(name="zpool", bufs=3, space="PSUM") as zpool, \
             tc.tile_pool(name="opool", bufs=2, space="PSUM") as opool, \
             tc.tile_pool(name="h1pool", bufs=6) as h1pool, \
             tc.tile_pool(name="h2pool", bufs=6) as h2pool, \
             tc.tile_pool(name="spool", bufs=4) as spool:

            # ---- constant + feats streams, all on the gpsimd SWDGE path,
            # emitted just-in-time in consumption order: compute emitted
            # after a DMA empirically waits for ALL prior SWDGE DMAs, so
            # nothing may be emitted earlier than needed ----
            feat_tiles = {}   # chunk -> (tile, col offset of chunk start)

            def fetch_super(s):
                # each super tile is a [67, range] bf16 tile assembled by two
                # SWDGE DMAs: the fp8 x rows cast in-flight to bf16 into
                # partitions 0:64, the bf16 coord rows into partitions 64:67
                c0, c1 = SUPERS[s]
                if s == 0:
                    t = cpool.tile([CIN + 1, WHDR + 4 * T], BF, name="s0t")
                    lo, hi = 0, WHDR + 4 * T
                else:
                    t = xpool.tile([CIN + 1, 8 * T], BF, name=f"sup{s}",
                                   tag="sup")
                    lo, hi = WHDR + c0 * T, WHDR + c1 * T
                nc.gpsimd.dma_start(out=t[0:C, 0:hi - lo],
                                    in_=x8.ap()[:, lo:hi])
                nc.gpsimd.dma_start(out=t[C:CIN + 1, 0:hi - lo],
                                    in_=xc.ap()[:, lo:hi])
                for k in range(c0, c1):
                    feat_tiles[k] = (t, (WHDR if s == 0 else 0) + (k - c0) * T)
                return t

            s0t = fetch_super(0)
            w0_sb = [s0t[:, 128 * p:128 * (p + 1)] for p in range(2)]

            wsb = cpool.tile([128, 1288], BF, name="wsb")
            fsb = cpool.tile([128, 3], F32, name="fsb")
            w2_sb = [wsb[:, 512 * p:512 * (p + 1)] for p in range(2)]
            w1_sb = [wsb[:, 1024 + 128 * p:1024 + 128 * (p + 1)]
                     for p in range(2)]
            b1_sb = [fsb[:, p:p + 1] for p in range(2)]

            def load_wpk():
                nc.gpsimd.dma_start(out=wsb, in_=wpk.ap())
                for k in range(3):
                    nc.vector.tensor_tensor(
                        out=fsb[:, k:k + 1],
                        in0=wsb[:, 1280 + 2 * k:1281 + 2 * k],
                        in1=wsb[:, 1281 + 2 * k:1282 + 2 * k],
                        op=AL.add)

            z1s, z2s, h1s, h2s = {}, {}, {}, {}
            obank = {"tile": None, "idx": -1}

            def flush_window(g):
                # window jj of bank b just completed (16 groups x 2 rows)
                b, jj = g // 64, (g % 64) // 16
                ob = obank["tile"]
                stage = spool.tile([32, 512], BF, name=f"st{g//16}", tag="st")
                nc.vector.tensor_scalar(out=stage,
                                        in0=ob[32 * jj:32 * jj + 32, :],
                                        scalar1=fsb[32 * jj:32 * jj + 32, 2:3],
                                        scalar2=None, op0=AL.add)
                # stage partition 8cb+4p+2s+m <-> instance 2p+m, position
                # (16b+4jj+cb)*T + s*512; one DMA per (p, s, m) keeps APs 2-D
                src = stage.rearrange("(cb p s m) f -> cb p s m f",
                                      p=2, s=2, m=2)
                # the final window flushes via gpsimd SWDGE (fast, and no
                # compute is emitted after it); the rest ride the sync ring
                eng = nc.gpsimd if g == GROUPS - 1 else nc.sync
                for par in range(2):
                    for s in range(2):
                        for m in range(2):
                            dst = bass.AP(
                                o,
                                (2 * par + m) * L + (16 * b + 4 * jj) * T + s * 512,
                                [[T, 4], [1, 512]])
                            eng.dma_start(out=dst, in_=src[:, par, s, m, :])

            # software-pipelined emission: iter i does L1(i), L3(i-2),
            # L2(i-1); relus follow their producers. DMAs are emitted at
            # the END of an iteration (after that L1) so earlier compute
            # never gates on them.
            for i in range(NCHUNK + 2):
                # L1(i): one [67,128] x [67,512] matmul per (pair, half);
                # feats rows 0:64 = x (cast from fp8), 64:67 = coords
                if i < NCHUNK:
                    ft, off = feat_tiles[i]
                    for p in range(2):
                        z1 = zpool.tile([128, T], F32, name=f"z1_{i}_{p}", tag="z")
                        for s in range(2):
                            nc.tensor.matmul(
                                z1[:, s * 512:(s + 1) * 512], w0_sb[p],
                                ft[:, off + s * 512:off + (s + 1) * 512],
                                start=True, stop=True)
                        z1s[(i, p)] = z1
                    for p in range(2):
                        h1 = h1pool.tile([128, T], BF, name=f"h1_{i}_{p}", tag="h1")
                        relu(h1, z1s.pop((i, p)), None, on_act=(p == 0))
                        h1s[(i, p)] = h1
                    feat_tiles.pop(i)

                if i == 0:
                    load_wpk()
                elif i in SUPER_EMIT:
                    fetch_super(SUPER_EMIT[i])

                # L2(i-1) before L3 so its relus (which free the z slots the
                # next L1 reuses, via the 3-slot FIFO rotation) are emitted
                # a full L3 segment earlier
                j2 = i - 1
                if 0 <= j2 < NCHUNK:
                    for p in range(2):
                        z2 = zpool.tile([128, T], F32, name=f"z2_{j2}_{p}", tag="z")
                        h1 = h1s.pop((j2, p))
                        for s in range(2):
                            nc.tensor.matmul(z2[:, s * 512:(s + 1) * 512], w1_sb[p],
                                             h1[:, s * 512:(s + 1) * 512],
                                             start=True, stop=True)
                        z2s[(j2, p)] = z2
                    for p in range(2):
                        h2 = h2pool.tile([128, T], BF, name=f"h2_{j2}_{p}", tag="h2")
                        relu(h2, z2s.pop((j2, p)), b1_sb[p], on_act=(p == 0))
                        h2s[(j2, p)] = h2

                # L3(i-2)
                j3 = i - 2
                if j3 >= 0:
                    for p in range(2):
                        h2 = h2s[(j3, p)]
                        for s in range(2):
                            g = j3 * 4 + p * 2 + s
                            lg = g % 64
                            if lg == 0:
                                obank["tile"] = opool.tile([128, 512], F32,
                                                           name=f"ob{g//64}",
                                                           tag="ob")
                                obank["idx"] = g // 64
                            jj, jv = lg // 16, lg % 16
                            nc.tensor.matmul(
                                obank["tile"][32 * jj:32 * jj + 32, :],
                                w2_sb[p][:, 32 * jv:32 * jv + 32],
                                h2[:, s * 512:(s + 1) * 512],
                                start=(jv == 0), stop=(jv == 15),
                                tile_position=(0, 32 * jj))
                            if jv == 15:
                                flush_window(g)
                        if j3 >= 1:
                            h2s.pop((j3 - 1, p), None)

    nc.compile()
    return nc


def _prep_inputs(x, mask_head_params, num_ins):
    x = np.asarray(x, dtype=np.float32)
    params = np.asarray(mask_head_params, dtype=np.float32)
    num_ins = np.asarray(num_ins)
    img_idx = np.repeat(np.arange(N_IMG), num_ins)
    assert img_idx.shape[0] == N_IMG * INS_PER_IMG

    loc_x = np.broadcast_to(np.arange(W, dtype=np.float32)[None, :], (H, W))
    loc_y = np.broadcast_to(np.arange(H, dtype=np.float32)[:, None], (H, W))
    coords = np.stack([loc_x.reshape(L), loc_y.reshape(L),
                       np.ones(L, np.float32)])          # [3, L]

    in_maps = []
    for c in range(N_CORES):
        inst = [4 * c + k for k in range(INST_PER_CORE)]
        imgs = {img_idx[q] for q in inst}
        assert len(imgs) == 1, "expected each core's instances on one image"
        img = img_idx[inst[0]]

        x8 = np.zeros((C, WHDR + L), dtype=f8e4)
        x8[:, WHDR:] = x[img].reshape(C, L).astype(f8e4)
        xc = np.zeros((3, WHDR + L), dtype=bf16)
        xc[0, WHDR:] = coords[0].astype(bf16)
        xc[1, WHDR:] = coords[1].astype(bf16)
        xc[2, WHDR:] = 1.0
        wpk = np.zeros((128, 1288), np.float32)
        fpk = np.zeros((128, 3), np.float32)  # f32 biases: b1p0 | b1p1 | b2

        for p in range(2):
            a, b = inst[2 * p], inst[2 * p + 1]
            w0_a = params[a, :L1].reshape(C, CIN)
            w0_b = params[b, :L1].reshape(C, CIN)
            b0_a = params[a, B1OFF:B1OFF + C]
            b0_b = params[b, B1OFF:B1OFF + C]
            for k, (wv, bv) in enumerate(((w0_a, b0_a), (w0_b, b0_b))):
                cols = slice(128 * p + 64 * k, 128 * p + 64 * k + 64)
                # coord lhsT rows: [w0[:,0]; w0[:,1]; b0]; x lhsT in fp8
                xc[0, cols] = wv[:, 0].astype(bf16)
                xc[1, cols] = wv[:, 1].astype(bf16)
                xc[2, cols] = bv.astype(bf16)
                x8[:, cols] = wv[:, 2:].T.astype(f8e4)

            w1_a = params[a, L1:L1 + L2_].reshape(C, C)
            w1_b = params[b, L1:L1 + L2_].reshape(C, C)
            w1tt = np.zeros((128, 128), np.float32)
            w1tt[:64, :64] = w1_a.T
            w1tt[64:, 64:] = w1_b.T
            wpk[:, 1024 + 128 * p:1024 + 128 * (p + 1)] = w1tt

            w2_a = params[a, L1 + L2_:L1 + L2_ + C]
            w2_b = params[b, L1 + L2_:L1 + L2_ + C]
            w2pair = np.zeros((128, 2), np.float32)
            w2pair[:64, 0] = w2_a
            w2pair[64:, 1] = w2_b
            w2pad = np.zeros((128, 512), np.float32)
            for j in range(16):
                w2pad[:, 32 * j + 2 * j:32 * j + 2 * j + 2] = w2pair
            wpk[:, 512 * p:512 * (p + 1)] = w2pad

            b1 = np.concatenate([params[a, B1OFF + C:B1OFF + 2 * C],
                                 params[b, B1OFF + C:B1OFF + 2 * C]])
            fpk[:, p] = b1

        # b2 per out-bank partition q: pair=((q//2)%4)//2, inst_in_pair=q%2
        for q in range(128):
            pair = ((q // 2) % 4) // 2
            mm = q % 2
            iid = inst[2 * pair + mm]
            fpk[q, 2] = params[iid, B1OFF + 2 * C] - MASK_BIAS_SHIFT
        # biases as bf16 (hi, lo) pairs appended to the weight pack
        hi = fpk.astype(bf16)
        lo = (fpk - hi.astype(np.float32)).astype(bf16)
        pk = wpk.astype(bf16)
        pk[:, 1280:1286:2] = hi
        pk[:, 1281:1287:2] = lo
        in_maps.append({"x8": x8, "xc": xc, "wpk": pk})
    return in_maps


def kernel(x, mask_head_params, num_ins):
    if "nc" not in _cache:
        _cache["nc"] = _build_program()
    nc = _cache["nc"]
    in_maps = _prep_inputs(x, mask_head_params, num_ins)
    res = run_bass_kernel_spmd(nc, in_maps, core_ids=list(range(N_CORES)))
    out = np.concatenate([r["o"].astype(np.float32) for r in res.results], axis=0)
    return out.reshape(1, N_IMG * INS_PER_IMG, H, W)


# revision 56
# speedup vs baseline: 12997.2354x; 1.0146x over previous
"""CondLaneHead DynamicMaskHead kernel for 8 Trainium2 NeuronCores.

Problem: per-instance 3-layer 1x1-conv MLP over a [64,160,256] feature map.
  feats = concat([loc_x, loc_y], x[img])            # [66, L], L = 160*256
  h1 = relu(w0 @ feats + b0)                        # [64, L]
  h2 = relu(w1 @ h1 + b1)                           # [64, L]
  out = w2 @ h2 + b2 - 2.19                         # [1, L]
32 instances (8 per image, 4 images). Sharding: 4 instances per core; each
core needs exactly one image's feature map.

Device mapping (per core, 4 instances = 2 pairs):
  - Layer 1 per pair is a pair of row-tiled matmuls that run CONCURRENTLY on
    the PE (disjoint 32-row groups) and accumulate into one PSUM tile:
      rows 0:64  : x part, fp8 e4m3 (weights and activations)
      rows 64:67 : [loc_x; loc_y; ones] affine part in bf16 (coords must
                   stay bf16: integers up to 255 are exact there)
    fp8 on the x rows halves the dominant HBM/DMA stream; its error lands
    in the coord-dominated h1 at ~0.1% relative.
  - Layer 2: block-diagonal [128, 128] bf16 weights, one matmul per pair.
  - Layer 3 (64->1): output packed across PSUM partitions. Matmuls write
    [32, 512] windows at partition bases 0/32/64/96 using zero-padded weight
    variants (w2 placed at columns 2j of window j), accumulating over 16
    position-groups per window; each completed window is bias-added and
    scatter-DMA'd immediately (fine-grained flush keeps the tail short).
  - DMA plan (the v1-v5 bottleneck): bulk feats ride the gpsimd SWDGE path
    (only path that sprays descriptors across all 16 SDMA engines,
    ~52-72 GB/s; the HWDGE rings drain at ~8 GB/s on one engine). Q7
    descriptor generation is ~70ns/descriptor and serializes, so the fp8
    x-weights ride as a 256-col header inside the first x-super (both are
    64-partition fp8) and the coord weights as a 256-col header inside the
    single coords DMA (3 descriptors for the whole image). Output windows
    flush in bf16 on the otherwise-idle sync HWDGE ring.
"""

import sys

if "/opt/trn_rl_repo" not in sys.path:
    sys.path.insert(0, "/opt/trn_rl_repo")

import numpy as np
import ml_dtypes

import concourse.bass as bass
import concourse.mybir as mybir
from concourse import bacc
from concourse.tile import TileContext
from concourse.bass_utils import run_bass_kernel_spmd

BF = mybir.dt.bfloat16
F32 = mybir.dt.float32
F8 = mybir.dt.float8e4
AT = mybir.ActivationFunctionType
AL = mybir.AluOpType
bf16 = ml_dtypes.bfloat16
f8e4 = ml_dtypes.float8_e4m3

# Problem geometry (hardcoded per spec)
N_IMG, INS_PER_IMG, C, H, W = 4, 8, 64, 160, 256
CIN = C + 2
L = H * W                       # 40960 positions
L1, L2_, L3_ = (C + 2) * C, C * C, C
B1OFF = L1 + L2_ + L3_          # offsets into the 8513-param vector
MASK_BIAS_SHIFT = 2.19

N_CORES = 8
INST_PER_CORE = 4               # 2 pairs
T = 1024                        # positions per chunk
NCHUNK = L // T                 # 40
WHDR = 256                      # weight-header cols in the x8/coords streams
GROUPS = NCHUNK * 4             # 160 [2, 512] position-groups per core
# graded x8 supers (chunk ranges); s0 also carries the fp8 weight header
SUPERS = [(0, 4), (4, 8), (8, 16), (16, 24), (24, 32), (32, 40)]
# super s emitted at END of loop iteration c0(s)-1 (latest point where no
# earlier chunk's compute is emitted after it -- any compute emitted after
# a SWDGE prep waits for that DMA's data)
SUPER_EMIT = {3: 1, 7: 2, 15: 3, 23: 4, 31: 5}

# relu op cost estimates (ns) for greedy ACT/DVE balancing
COST_DVE = (120 + T) / 0.96
COST_ACT = (352 + T) / 1.2

_cache = {}


def _build_program():
    nc = bacc.Bacc("TRN2", target_bir_lowering=False, debug=False)

    # x rows in fp8 e4m3, with the per-pair L1 x-weights as a 256-col header
    x8 = nc.dram_tensor("x8", [C, WHDR + L], F8, kind="ExternalInput")
    # [loc_x; loc_y; ones] rows in bf16, with the per-pair coord-weight
    # lhsT ([w0[:,0]; w0[:,1]; b0] x 128 outs) as a 256-col header
    xc = nc.dram_tensor("xc", [3, WHDR + L], BF, kind="ExternalInput")
    # packed constants: [0:512] w2p0 | [512:1024] w2p1 | [1024:1152] w1t0 |
    # [1152:1280] w1t1 | [1280:1286] f32 biases as bf16 (hi, lo) pairs:
    # b1p0, b1p1, b2
    wpk = nc.dram_tensor("wpk", [128, 1288], BF, kind="ExternalInput")
    o = nc.dram_tensor("o", [INST_PER_CORE, L], BF, kind="ExternalOutput")

    def relu(dst, src, bias_ap, on_act):
        # Engine is chosen by slot criticality, not greedy balance: the
        # 3-slot PSUM FIFO rotation reuses z1p0/z2p0's slots soonest, so
        # those relus go to the lower-latency, less-loaded ACT engine.
        if on_act:
            if bias_ap is None:
                nc.scalar.activation(dst, src, AT.Relu)
            else:
                nc.scalar.activation(dst, src, AT.Relu, bias=bias_ap)
        else:
            if bias_ap is None:
                nc.vector.tensor_scalar(out=dst, in0=src, scalar1=0.0,
                                        scalar2=None, op0=AL.max)
            else:
                nc.vector.tensor_scalar(out=dst, in0=src, scalar1=bias_ap,
                                        scalar2=0.0, op0=AL.add, op1=AL.max)

    with TileContext(nc) as tc:
        with tc.tile_pool(name="consts", bufs=1) as cpool, \
             tc.tile_pool(name="xpool", bufs=3) as xpool, \
             tc.tile_pool(name="zpool", bufs=3, space="PSUM") as zpool, \
             tc.tile_pool(name="opool", bufs=2, space="PSUM") as opool, \
             tc.tile_pool(name="h1pool", bufs=6) as h1pool, \
             tc.tile_pool(name="h2pool", bufs=6) as h2pool, \
             tc.tile_pool(name="spool", bufs=4) as spool:

            # ---- constant + feats streams, all on the gpsimd SWDGE path,
            # emitted just-in-time in consumption order: compute emitted
            # after a DMA empirically waits for ALL prior SWDGE DMAs, so
            # nothing may be emitted earlier than needed ----
            feat_tiles = {}   # chunk -> (tile, col offset of chunk start)

            def fetch_super(s):
                # each super tile is a [67, range] bf16 tile assembled by two
                # SWDGE DMAs: the fp8 x rows cast in-flight to bf16 into
                # partitions 0:64, the bf16 coord rows into partitions 64:67
                c0, c1 = SUPERS[s]
                if s == 0:
                    t = cpool.tile([CIN + 1, WHDR + 4 * T], BF, name="s0t")
                    lo, hi = 0, WHDR + 4 * T
                else:
                    t = xpool.tile([CIN + 1, 8 * T], BF, name=f"sup{s}",
                                   tag="sup")
                    lo, hi = WHDR + c0 * T, WHDR + c1 * T
                nc.gpsimd.dma_start(out=t[0:C, 0:hi - lo],
                                    in_=x8.ap()[:, lo:hi])
                nc.gpsimd.dma_start(out=t[C:CIN + 1, 0:hi - lo],
                                    in_=xc.ap()[:, lo:hi])
                for k in range(c0, c1):
                    feat_tiles[k] = (t, (WHDR if s == 0 else 0) + (k - c0) * T)
                return t

            s0t = fetch_super(0)
            w0_sb = [s0t[:, 128 * p:128 * (p + 1)] for p in range(2)]

            wsb = cpool.tile([128, 1288], BF, name="wsb")
            fsb = cpool.tile([128, 3], F32, name="fsb")
            w2_sb = [wsb[:, 512 * p:512 * (p + 1)] for p in range(2)]
            w1_sb = [wsb[:, 1024 + 128 * p:1024 + 128 * (p + 1)]
                     for p in range(2)]
            b1_sb = [fsb[:, p:p + 1] for p in range(2)]

            def load_wpk():
                nc.gpsimd.dma_start(out=wsb, in_=wpk.ap())
                for k in range(3):
                    nc.vector.tensor_tensor(
                        out=fsb[:, k:k + 1],
                        in0=wsb[:, 1280 + 2 * k:1281 + 2 * k],
                        in1=wsb[:, 1281 + 2 * k:1282 + 2 * k],
                        op=AL.add)

            z1s, z2s, h1s, h2s = {}, {}, {}, {}
            obank = {"tile": None, "idx": -1}

            def flush_window(g):
                # window jj of bank b just completed (16 groups x 2 rows)
                b, jj = g // 64, (g % 64) // 16
                ob = obank["tile"]
                stage = spool.tile([32, 512], BF, name=f"st{g//16}", tag="st")
                nc.vector.tensor_scalar(out=stage,
                                        in0=ob[32 * jj:32 * jj + 32, :],
                                        scalar1=fsb[32 * jj:32 * jj + 32, 2:3],
                                        scalar2=None, op0=AL.add)
                # stage partition 8cb+4p+2s+m <-> instance 2p+m, position
                # (16b+4jj+cb)*T + s*512; one DMA per (p, s, m) keeps APs 2-D
                src = stage.rearrange("(cb p s m) f -> cb p s m f",
                                      p=2, s=2, m=2)
                eng = nc.sync
                for par in range(2):
                    for s in range(2):
                        for m in range(2):
                            dst = bass.AP(
                                o,
                                (2 * par + m) * L + (16 * b + 4 * jj) * T + s * 512,
                                [[T, 4], [1, 512]])
                            eng.dma_start(out=dst, in_=src[:, par, s, m, :])

            # software-pipelined emission: iter i does L1(i), L3(i-2),
            # L2(i-1); relus follow their producers. DMAs are emitted at
            # the END of an iteration (after that L1) so earlier compute
            # never gates on them.
            for i in range(NCHUNK + 2):
                # L1(i): one [67,128] x [67,512] matmul per (pair, half);
                # feats rows 0:64 = x (cast from fp8), 64:67 = coords
                if i < NCHUNK:
                    ft, off = feat_tiles[i]
                    for p in range(2):
                        z1 = zpool.tile([128, T], F32, name=f"z1_{i}_{p}", tag="z")
                        for s in range(2):
                            nc.tensor.matmul(
                                z1[:, s * 512:(s + 1) * 512], w0_sb[p],
                                ft[:, off + s * 512:off + (s + 1) * 512],
                                start=True, stop=True)
                        z1s[(i, p)] = z1
                    for p in range(2):
                        h1 = h1pool.tile([128, T], BF, name=f"h1_{i}_{p}", tag="h1")
                        relu(h1, z1s.pop((i, p)), None, on_act=(p == 0))
                        h1s[(i, p)] = h1
                    feat_tiles.pop(i)

                if i == 0:
                    load_wpk()
                elif i in SUPER_EMIT:
                    fetch_super(SUPER_EMIT[i])

                # L2(i-1) before L3 so its relus (which free the z slots the
                # next L1 reuses, via the 3-slot FIFO rotation) are emitted
                # a full L3 segment earlier
                j2 = i - 1
                if 0 <= j2 < NCHUNK:
                    for p in range(2):
                        z2 = zpool.tile([128, T], F32, name=f"z2_{j2}_{p}", tag="z")
                        h1 = h1s.pop((j2, p))
                        for s in range(2):
                            nc.tensor.matmul(z2[:, s * 512:(s + 1) * 512], w1_sb[p],
                                             h1[:, s * 512:(s + 1) * 512],
                                             start=True, stop=True)
                        z2s[(j2, p)] = z2
                    for p in range(2):
                        h2 = h2pool.tile([128, T], BF, name=f"h2_{j2}_{p}", tag="h2")
                        z2 = z2s.pop((j2, p))
                        if j2 == NCHUNK - 1:
                            # tail drain: split across both engines to halve
                            # the last relu latency before the final L3+flush
                            nc.scalar.activation(h2[:, 0:512], z2[:, 0:512],
                                                 AT.Relu, bias=b1_sb[p])
                            nc.vector.tensor_scalar(
                                out=h2[:, 512:1024], in0=z2[:, 512:1024],
                                scalar1=b1_sb[p], scalar2=0.0,
                                op0=AL.add, op1=AL.max)
                        else:
                            relu(h2, z2, b1_sb[p], on_act=(p == 0))
                        h2s[(j2, p)] = h2

                # L3(i-2)
                j3 = i - 2
                if j3 >= 0:
                    for p in range(2):
                        h2 = h2s[(j3, p)]
                        for s in range(2):
                            g = j3 * 4 + p * 2 + s
                            lg = g % 64
                            if lg == 0:
                                obank["tile"] = opool.tile([128, 512], F32,
                                                           name=f"ob{g//64}",
                                                           tag="ob")
                                obank["idx"] = g // 64
                            jj, jv = lg // 16, lg % 16
                            nc.tensor.matmul(
                                obank["tile"][32 * jj:32 * jj + 32, :],
                                w2_sb[p][:, 32 * jv:32 * jv + 32],
                                h2[:, s * 512:(s + 1) * 512],
                                start=(jv == 0), stop=(jv == 15),
                                tile_position=(0, 32 * jj))
                            if jv == 15:
                                flush_window(g)
                        if j3 >= 1:
                            h2s.pop((j3 - 1, p), None)

    nc.compile()
    return nc


def _prep_inputs(x, mask_head_params, num_ins):
    x = np.asarray(x, dtype=np.float32)
    params = np.asarray(mask_head_params, dtype=np.float32)
    num_ins = np.asarray(num_ins)
    img_idx = np.repeat(np.arange(N_IMG), num_ins)
    assert img_idx.shape[0] == N_IMG * INS_PER_IMG

    loc_x = np.broadcast_to(np.arange(W, dtype=np.float32)[None, :], (H, W))
    loc_y = np.broadcast_to(np.arange(H, dtype=np.float32)[:, None], (H, W))
    coords = np.stack([loc_x.reshape(L), loc_y.reshape(L),
                       np.ones(L, np.float32)])          # [3, L]

    in_maps = []
    for c in range(N_CORES):
        inst = [4 * c + k for k in range(INST_PER_CORE)]
        imgs = {img_idx[q] for q in inst}
        assert len(imgs) == 1, "expected each core's instances on one image"
        img = img_idx[inst[0]]

        x8 = np.zeros((C, WHDR + L), dtype=f8e4)
        x8[:, WHDR:] = x[img].reshape(C, L).astype(f8e4)
        xc = np.zeros((3, WHDR + L), dtype=bf16)
        xc[0, WHDR:] = coords[0].astype(bf16)
        xc[1, WHDR:] = coords[1].astype(bf16)
        xc[2, WHDR:] = 1.0
        wpk = np.zeros((128, 1288), np.float32)
        fpk = np.zeros((128, 3), np.float32)  # f32 biases: b1p0 | b1p1 | b2

        for p in range(2):
            a, b = inst[2 * p], inst[2 * p + 1]
            w0_a = params[a, :L1].reshape(C, CIN)
            w0_b = params[b, :L1].reshape(C, CIN)
            b0_a = params[a, B1OFF:B1OFF + C]
            b0_b = params[b, B1OFF:B1OFF + C]
            for k, (wv, bv) in enumerate(((w0_a, b0_a), (w0_b, b0_b))):
                cols = slice(128 * p + 64 * k, 128 * p + 64 * k + 64)
                # coord lhsT rows: [w0[:,0]; w0[:,1]; b0]; x lhsT in fp8
                xc[0, cols] = wv[:, 0].astype(bf16)
                xc[1, cols] = wv[:, 1].astype(bf16)
                xc[2, cols] = bv.astype(bf16)
                x8[:, cols] = wv[:, 2:].T.astype(f8e4)

            w1_a = params[a, L1:L1 + L2_].reshape(C, C)
            w1_b = params[b, L1:L1 + L2_].reshape(C, C)
            w1tt = np.zeros((128, 128), np.float32)
            w1tt[:64, :64] = w1_a.T
            w1tt[64:, 64:] = w1_b.T
            wpk[:, 1024 + 128 * p:1024 + 128 * (p + 1)] = w1tt

            w2_a = params[a, L1 + L2_:L1 + L2_ + C]
            w2_b = params[b, L1 + L2_:L1 + L2_ + C]
            w2pair = np.zeros((128, 2), np.float32)
            w2pair[:64, 0] = w2_a
            w2pair[64:, 1] = w2_b
            w2pad = np.zeros((128, 512), np.float32)
            for j in range(16):
                w2pad[:, 32 * j + 2 * j:32 * j + 2 * j + 2] = w2pair
            wpk[:, 512 * p:512 * (p + 1)] = w2pad

            b1 = np.concatenate([params[a, B1OFF + C:B1OFF + 2 * C],
                                 params[b, B1OFF + C:B1OFF + 2 * C]])
            fpk[:, p] = b1

        # b2 per out-bank partition q: pair=((q//2)%4)//2, inst_in_pair=q%2
        for q in range(128):
            pair = ((q // 2) % 4) // 2
            mm = q % 2
            iid = inst[2 * pair + mm]
            fpk[q, 2] = params[iid, B1OFF + 2 * C] - MASK_BIAS_SHIFT
        # biases as bf16 (hi, lo) pairs appended to the weight pack
        hi = fpk.astype(bf16)
        lo = (fpk - hi.astype(np.float32)).astype(bf16)
        pk = wpk.astype(bf16)
        pk[:, 1280:1286:2] = hi
        pk[:, 1281:1287:2] = lo
        in_maps.append({"x8": x8, "xc": xc, "wpk": pk})
    return in_maps


def kernel(x, mask_head_params, num_ins):
    if "nc" not in _cache:
        _cache["nc"] = _build_program()
    nc = _cache["nc"]
    in_maps = _prep_inputs(x, mask_head_params, num_ins)
    res = run_bass_kernel_spmd(nc, in_maps, core_ids=list(range(N_CORES)))
    out = np.concatenate([r["o"].astype(np.float32) for r in res.results], axis=0)
    return out.reshape(1, N_IMG * INS_PER_IMG, H, W)
